# revision 1
# baseline (speedup 1.0000x reference)
"""AttentionPooling (global-softmax segment-sum) Trainium2 Bass kernel.

  scores = x @ W + b ; attn = softmax(scores, axis=0) ; out = segment_sum(x*attn, batch, G)

Design (8 cores, SPMD, raw Bass — no Tile: this walrus build allows only ONE
semaphore wait per instruction, so every cross-engine wait is its own wait_ge):

 * softmax is shift-invariant => b drops out; fixed shift M=0 (scores~N(0,1)).
 * device computes, per core, the unnormalized pooled_g = sum_{i in g} e^{s_i} x_i
   and Z_core = sum_i e^{s_i}; host divides by Z = sum Z_core at the end.
 * shard by SEGMENT BLOCKS: G segs -> cores x blocks x 128 segs. batch is sorted
   => each block's nodes are a contiguous node range; host zero-pads every block
   to one fixed node budget (multiple of 2048) so all 8 cores run the same
   static SPMD program. Pads: x=0 => e^0=1 pollutes Z only (host subtracts the
   pad count); pad batchloc=999 never matches the one-hot => pooled unpolluted.
 * per 4096-node super-chunk s (node n: partition p=n%128, chunk c=n//128):
     SYNC  dma xt[s] <- host-preswizzled bf16 [128, 32x128]
     DVE   xw = xt * Wrep (bf16 TT, 2x mode); tree-fold d 128->8 with 2x TT
           adds (tensor_reduce only runs 1x), then one small 1x reduce ->
           scores[:, 32] f32
     ACT   expw[:,32] = Exp(scores) (accum_out -> Z partial column)
     one-hot per chunk A[p,j] = (iota_j==batchloc_p)*expw_p, split three ways:
       kd chunks on DVE (tensor_scalar is_equal+mult, 4x mode, ~94ns)
       kg chunks on GPSIMD (same op, ~273ns)
       ka chunks on ACT: u=(iota-bl)^2 [Square, bias=-bl]; A=Relu(expw-u*expw)
         [Relu, scale=-expw, bias=+expw] — exact for integer iota/bl.
     PE    psum[128 segs, 128 d] += A.T @ x_chunk (bf16 matmul, 1 cyc/row)
   one-hot work of super s-1 overlaps scores of super s (software pipeline).
 * per 128-seg block: ACT copies psum->sbuf stage; one DMA out at the end.
 * blocks are ragged: blk_ch chunks (e.g. 125 = 32+32+32+29 supers); short
   supers shed DVE one-hot chunks first (DVE is the busiest engine).
 * per-block output slices DMA out as soon as staged (overlaps the tail).
 * TimelineSim (cost model): ~364 us/core; DMA floor ~197 us; engines
   DVE ~340 / Pool ~335 / ACT ~300 / PE ~110 us. Measured full-size
   relative error vs fp32 reference: 0.0059 (bf16 data path).
"""

import os
import numpy as np
import ml_dtypes

import concourse.bass as bass
import concourse.mybir as mybir
from concourse.bass_utils import run_bass_kernel_spmd

BF16 = mybir.dt.bfloat16
F32 = mybir.dt.float32
ALU = mybir.AluOpType
ACTF = mybir.ActivationFunctionType

N_CORES = 8
D = 128
P = 128
SUP_CH = 32            # chunks per super-chunk
SUP = P * SUP_CH       # 2048 nodes per super-chunk
NXB = 12               # x-tile buffer depth (one DMA in flight per slot)
NAT = 10               # one-hot tile slots per producing engine

_prog_cache = {}


def _build(blocks, blk_ch, kd, ka):
    """blocks 128-seg blocks/core; blk_ch = chunks per block (ragged: supers of
    <=SUP_CH chunks); one-hot split per super: kd on DVE, ka on ACT, rest GPSIMD."""
    # per-super chunk counts, uniform across blocks (SPMD)
    sup_shape = []
    r = blk_ch
    while r > 0:
        t = min(SUP_CH, r)
        sup_shape.append(t)
        r -= t
    spb = len(sup_shape)
    nsup = blocks * spb
    ch_of = [sup_shape[s % spb] for s in range(nsup)]
    CH0 = [0]
    for s in range(nsup):
        CH0.append(CH0[-1] + ch_of[s])
    nch = CH0[-1]
    # per-super split (smaller supers shed GPSIMD chunks first, then ACT, DVE)
    kd_of, ka_of, kg_of = [], [], []
    kg_full = SUP_CH - kd - ka
    for s in range(nsup):
        n = ch_of[s]
        # short supers shed DVE chunks first (DVE is the busiest engine)
        g = min(kg_full, n); a = min(ka, n - g); d = n - g - a
        kd_of.append(d); ka_of.append(a); kg_of.append(g)
    nc = bass.Bass()

    xp_h = nc.declare_dram_parameter("xp", [nch * P * D], BF16, isOutput=False)
    bl_h = nc.declare_dram_parameter("bl", [P, nch], F32, isOutput=False)
    wrep_h = nc.declare_dram_parameter("wrep", [P, SUP_CH * D], BF16, isOutput=False)
    iota_h = nc.declare_dram_parameter("iota", [P, P], BF16, isOutput=False)
    bln_h = nc.declare_dram_parameter("bln", [P, nch], F32, isOutput=False)
    out_h = nc.declare_dram_parameter("outp", [P, blocks * D], F32, isOutput=True)
    z_h = nc.declare_dram_parameter("zout", [P, 1], F32, isOutput=True)

    # tick tables (pass 1: pure counting in emission order) -----------------
    # DVE iter s: TT(s)(+1), folds(0), RED(s)(+1) [s<nsup]; then kd_of[s-1] TS (+1 each)
    T_DVE_TT, T_DVE_RED, T_DVE_TS = {}, {}, {}
    t = 0
    for s in range(nsup + 1):
        if s < nsup:
            t += 1; T_DVE_TT[s] = t
            t += 1; T_DVE_RED[s] = t
        if s >= 1:
            for i in range(kd_of[s - 1]):
                t += 1; T_DVE_TS[(s - 1, i)] = t
    zred_tick = t + 1
    # GPSIMD iter s>=1: kg_of[s-1] TS
    T_GP_TS = {}
    t = 0
    for s in range(1, nsup + 1):
        for i in range(kg_of[s - 1]):
            t += 1; T_GP_TS[(s - 1, i)] = t
    # ACT iter s: Exp(+1), negate(0) [s<nsup]; ka_of[s-1] pairs (+1 each on Relu)
    T_ACT_EXP, T_ACT_OH = {}, {}
    t = 0
    for s in range(nsup + 1):
        if s < nsup:
            t += 1; T_ACT_EXP[s] = t
        if s >= 1:
            for j in range(ka_of[s - 1]):
                t += 1; T_ACT_OH[(s - 1, j)] = t
    # PE: one mm per chunk, supers in order
    T_PE_MM = {}
    t = 0
    for s in range(nsup):
        for c in range(ch_of[s]):
            t += 1; T_PE_MM[(s, c)] = t

    def t_dve_tt(s):
        return T_DVE_TT[s]

    def t_dve_red(s):
        return T_DVE_RED[s]

    def t_dve_ts(sm1, i):
        return T_DVE_TS[(sm1, i)]

    def t_gp_ts(sm1, i):
        return T_GP_TS[(sm1, i)]

    def t_act_exp(s):
        return T_ACT_EXP[s]

    def t_act_oh(sm1, j):
        return T_ACT_OH[(sm1, j)]

    def t_pe_mm(s, c):
        return T_PE_MM[(s, c)]

    DVE_LIST = [(s, i) for s in range(nsup) for i in range(kd_of[s])]
    GP_LIST = [(s, kd_of[s] + i) for s in range(nsup) for i in range(kg_of[s])]
    ACT_LIST = [(s, kd_of[s] + kg_of[s] + j) for s in range(nsup)
                for j in range(ka_of[s])]
    DVE_IDX0 = [0]
    for s in range(nsup):
        DVE_IDX0.append(DVE_IDX0[-1] + kd_of[s])
    GP_IDX0 = [0]
    for s in range(nsup):
        GP_IDX0.append(GP_IDX0[-1] + kg_of[s])
    ACT_IDX0 = [0]
    for s in range(nsup):
        ACT_IDX0.append(ACT_IDX0[-1] + ka_of[s])

    import contextlib
    with contextlib.ExitStack() as ctx:
        sem_xc = ctx.enter_context(nc.semaphore("sem_xc"))
        sem_x = [ctx.enter_context(nc.semaphore(f"sem_x{j}")) for j in range(NXB)]
        sem_dve = ctx.enter_context(nc.semaphore("sem_dve"))
        sem_act = ctx.enter_context(nc.semaphore("sem_act"))
        sem_gp = ctx.enter_context(nc.semaphore("sem_gp"))
        sem_pe = ctx.enter_context(nc.semaphore("sem_pe"))
        sem_cp = ctx.enter_context(nc.semaphore("sem_cp"))
        sem_out = ctx.enter_context(nc.semaphore("sem_out"))

        wrep_t = ctx.enter_context(nc.sbuf_tensor([P, SUP_CH * D], BF16))
        iota_t = ctx.enter_context(nc.sbuf_tensor([P, P], BF16))
        bl_t = ctx.enter_context(nc.sbuf_tensor([P, nch], F32))
        xt = [ctx.enter_context(nc.sbuf_tensor(f"xt{j}", [P, SUP_CH * D], BF16))
              for j in range(NXB)]
        xw_t = ctx.enter_context(nc.sbuf_tensor([P, SUP_CH * D], BF16))
        scores_t = ctx.enter_context(nc.sbuf_tensor([P, nch], F32))
        expw_t = ctx.enter_context(nc.sbuf_tensor([P, nch], F32))
        zc_t = ctx.enter_context(nc.sbuf_tensor([P, nsup], F32))
        zsum_t = ctx.enter_context(nc.sbuf_tensor([P, 1], F32))
        stage_t = ctx.enter_context(nc.sbuf_tensor([P, blocks * D], F32))
        atd = [ctx.enter_context(nc.sbuf_tensor(f"atd{j}", [P, P], BF16)) for j in range(NAT)]
        atg = [ctx.enter_context(nc.sbuf_tensor(f"atg{j}", [P, P], BF16)) for j in range(NAT)]
        ata = [ctx.enter_context(nc.sbuf_tensor(f"ata{j}", [P, P], BF16)) for j in range(NAT)]
        uat = ctx.enter_context(nc.sbuf_tensor("uat", [P, P], BF16))
        bln_t = ctx.enter_context(nc.sbuf_tensor("bln_t", [P, nch], F32))
        nexpw_t = ctx.enter_context(nc.sbuf_tensor("nexpw_t", [P, nch], F32))
        pt = [ctx.enter_context(nc.psum_tensor(f"pt{j}", [P, 512], F32)) for j in range(2)]


        with nc.Block() as block:

            @block.sync
            def _(sync):
                sync.dma_start(out=wrep_t[:], in_=wrep_h[:]).then_inc(sem_xc, 16)
                sync.dma_start(out=iota_t[:], in_=iota_h[:]).then_inc(sem_xc, 16)
                sync.dma_start(out=bl_t[:], in_=bl_h[:]).then_inc(sem_xc, 16)
                sync.dma_start(out=bln_t[:], in_=bln_h[:]).then_inc(sem_xc, 16)
                for s in range(nsup):
                    j = s % NXB
                    ch = ch_of[s]
                    if s >= NXB:
                        so = s - NXB  # slot's previous super: consumers done?
                        sync.wait_ge(sem_dve, t_dve_tt(so))
                        sync.wait_ge(sem_pe, t_pe_mm(so, ch_of[so] - 1))
                    sync.dma_start(
                        out=xt[j][:, 0:ch * D].rearrange("p (c d) -> p c d", d=D),
                        in_=xp_h[CH0[s] * P * D:CH0[s + 1] * P * D].rearrange(
                            "(p c d) -> p c d", p=P, d=D),
                    ).then_inc(sem_x[j], 16)
                # outputs: stream each block's slice as soon as it is staged
                for b in range(blocks):
                    sync.wait_ge(sem_cp, b + 1)
                    sync.dma_start(
                        out=out_h[:, b * D:(b + 1) * D],
                        in_=stage_t[:, b * D:(b + 1) * D],
                    ).then_inc(sem_out, 16)
                sync.wait_ge(sem_dve, zred_tick)
                sync.dma_start(out=z_h[:], in_=zsum_t[:]).then_inc(sem_out, 16)
                sync.wait_ge(sem_out, 16 * (blocks + 1))

            @block.vector
            def _(vector):
                vector.wait_ge(sem_xc, 64)
                for s in range(nsup + 1):
                    if s < nsup:
                        j = s % NXB
                        ch = ch_of[s]
                        vector.wait_ge(sem_x[j], 16 * (s // NXB + 1))
                        nc.vector.tensor_tensor(
                            out=xw_t[:, 0:ch * D], in0=xt[j][:, 0:ch * D],
                            in1=wrep_t[:, 0:ch * D], op=ALU.mult
                        ).then_inc(sem_dve, 1)
                        # tree-fold the d-axis 128->8 with 2x-mode TT adds
                        # (tensor_reduce runs at 1x; folds are ~1.6x cheaper),
                        # then one small 1x reduce. bf16 partials cost ~0.5%
                        # extra score error — fine at the 2e-2 scale.
                        xw3 = xw_t[:, 0:ch * D].rearrange("p (c d) -> p c d", d=D)
                        for w in (64, 32, 16, 8):
                            nc.vector.tensor_tensor(
                                out=xw3[:, :, 0:w], in0=xw3[:, :, 0:w],
                                in1=xw3[:, :, w:2 * w], op=ALU.add,
                            )
                        nc.vector.tensor_reduce(
                            out=scores_t[:, CH0[s]:CH0[s + 1]],
                            in_=xw3[:, :, 0:8],
                            axis=mybir.AxisListType.X, op=ALU.add,
                        ).then_inc(sem_dve, 1)
                    if s >= 1 and kd_of[s - 1] > 0:
                        sm1 = s - 1
                        vector.wait_ge(sem_act, t_act_exp(sm1))
                        for i in range(kd_of[sm1]):
                            gd = DVE_IDX0[sm1] + i
                            if gd >= NAT:  # one-hot slot: wait mm that freed it
                                po, io = DVE_LIST[gd - NAT]
                                vector.wait_ge(sem_pe, t_pe_mm(po, io))
                            ca = CH0[sm1] + i
                            nc.vector.tensor_scalar(
                                atd[gd % NAT][:], iota_t[:],
                                bl_t[:, ca:ca + 1], expw_t[:, ca:ca + 1],
                                ALU.is_equal, ALU.mult,
                            ).then_inc(sem_dve, 1)
                # Z final reduction
                vector.wait_ge(sem_act, t_act_exp(nsup - 1))
                nc.vector.tensor_reduce(
                    out=zsum_t[:], in_=zc_t[:],
                    axis=mybir.AxisListType.X, op=ALU.add,
                ).then_inc(sem_dve, 1)

            @block.gpsimd
            def _(gpsimd):
                gpsimd.wait_ge(sem_xc, 64)
                for s in range(1, nsup + 1):
                    sm1 = s - 1
                    if kg_of[sm1] == 0:
                        continue
                    gpsimd.wait_ge(sem_act, t_act_exp(sm1))
                    for i in range(kg_of[sm1]):
                        gg = GP_IDX0[sm1] + i
                        if gg >= NAT:
                            po, co = GP_LIST[gg - NAT]
                            gpsimd.wait_ge(sem_pe, t_pe_mm(po, co))
                        ca = CH0[sm1] + kd_of[sm1] + i
                        nc.gpsimd.tensor_scalar(
                            atg[gg % NAT][:], iota_t[:],
                            bl_t[:, ca:ca + 1], expw_t[:, ca:ca + 1],
                            ALU.is_equal, ALU.mult,
                        ).then_inc(sem_gp, 1)

            @block.scalar
            def _(scalar):
                scalar.wait_ge(sem_xc, 64)
                for s in range(nsup + 1):
                    if s < nsup:
                        scalar.wait_ge(sem_dve, t_dve_red(s))
                        nc.scalar.activation(
                            out=expw_t[:, CH0[s]:CH0[s + 1]],
                            in_=scores_t[:, CH0[s]:CH0[s + 1]],
                            func=ACTF.Exp,
                            accum_out=zc_t[:, s:s + 1],
                        ).then_inc(sem_act, 1)
                        if ka > 0:
                            nc.scalar.activation(
                                out=nexpw_t[:, CH0[s]:CH0[s + 1]],
                                in_=expw_t[:, CH0[s]:CH0[s + 1]],
                                func=ACTF.Copy, scale=-1.0,
                            )
                    if s >= 1 and ka_of[s - 1] > 0:
                        sm1 = s - 1
                        for j in range(ka_of[sm1]):
                            ga = ACT_IDX0[sm1] + j
                            if ga >= NAT:
                                po, co = ACT_LIST[ga - NAT]
                                scalar.wait_ge(sem_pe, t_pe_mm(po, co))
                            ca = CH0[sm1] + kd_of[sm1] + kg_of[sm1] + j
                            # u = (iota - bl)^2 ; A = Relu(expw*(1 - u))
                            nc.scalar.activation(
                                out=uat[:], in_=iota_t[:], func=ACTF.Square,
                                bias=bln_t[:, ca:ca + 1], scale=1.0,
                            )
                            nc.scalar.activation(
                                out=ata[ga % NAT][:], in_=uat[:], func=ACTF.Relu,
                                bias=expw_t[:, ca:ca + 1],
                                scale=nexpw_t[:, ca:ca + 1],
                            ).then_inc(sem_act, 1)
                    if s >= 1 and (s - 1) % spb == spb - 1:
                        b = (s - 1) // spb
                        sl = b * spb + spb - 1
                        scalar.wait_ge(sem_pe, t_pe_mm(sl, ch_of[sl] - 1))
                        nc.scalar.copy(
                            out=stage_t[:, b * D:(b + 1) * D], in_=pt[b % 2][:, 0:D]
                        ).then_inc(sem_cp, 1)

            @block.tensor
            def _(tensor):
                for sm1 in range(nsup):
                    b = sm1 // spb
                    j = sm1 % NXB
                    tensor.wait_ge(sem_x[j], 16 * (sm1 // NXB + 1))
                    if sm1 % spb == 0 and b >= 2:
                        tensor.wait_ge(sem_cp, b - 1)
                    for c in range(ch_of[sm1]):
                        if c < kd_of[sm1]:
                            tensor.wait_ge(sem_dve, t_dve_ts(sm1, c))
                            a = atd[(DVE_IDX0[sm1] + c) % NAT]
                        elif c < kd_of[sm1] + kg_of[sm1]:
                            i = c - kd_of[sm1]
                            tensor.wait_ge(sem_gp, t_gp_ts(sm1, i))
                            a = atg[(GP_IDX0[sm1] + i) % NAT]
                        else:
                            jx = c - kd_of[sm1] - kg_of[sm1]
                            tensor.wait_ge(sem_act, t_act_oh(sm1, jx))
                            a = ata[(ACT_IDX0[sm1] + jx) % NAT]
                        nc.tensor.matmul(
                            pt[b % 2][:, 0:D],
                            lhsT=a[:],
                            rhs=xt[j][:, c * D:(c + 1) * D],
                            start=(sm1 % spb == 0 and c == 0),
                            stop=(sm1 % spb == spb - 1 and c == ch_of[sm1] - 1),
                        ).then_inc(sem_pe, 1)

    return nc


def _pool(x, batch, W, num_graphs, n_cores=N_CORES, kd=None, ka=None):
    n = x.shape[0]
    segs_per_core = num_graphs // n_cores
    blocks = segs_per_core // P

    seg_starts = np.searchsorted(batch, np.arange(0, num_graphs + 1, P))
    blk_cnt = np.diff(seg_starts)
    blk_ch = max(1, int(np.ceil(blk_cnt.max() / P)))    # chunks per block
    n_b = blk_ch * P
    nch = blocks * blk_ch
    L = blocks * n_b
    sup_shape = []
    r = blk_ch
    while r > 0:
        t = min(SUP_CH, r)
        sup_shape.append(t)
        r -= t
    spb = len(sup_shape)
    nsup = blocks * spb
    if kd is None:
        kd = int(os.environ.get("KD", "7"))
    if ka is None:
        ka = int(os.environ.get("KA", "6"))

    x_bf = np.ascontiguousarray(x).astype(ml_dtypes.bfloat16)
    bloc_all = (batch % P).astype(np.float32)

    wrep = np.tile(np.asarray(W, np.float32).reshape(1, D), (P, SUP_CH)).astype(
        ml_dtypes.bfloat16)
    iota = np.broadcast_to(np.arange(P, dtype=np.float32), (P, P)).astype(
        ml_dtypes.bfloat16)

    in_maps, pad_counts = [], []
    for core in range(n_cores):
        xflat = np.zeros((L, D), ml_dtypes.bfloat16)
        blflat = np.full((L,), 999.0, np.float32)
        for bi in range(blocks):
            gb = core * blocks + bi
            s0, s1 = seg_starts[gb], seg_starts[gb + 1]
            cnt = s1 - s0
            xflat[bi * n_b: bi * n_b + cnt] = x_bf[s0:s1]
            blflat[bi * n_b: bi * n_b + cnt] = bloc_all[s0:s1]
        slabs = []
        off = 0
        for s in range(nsup):
            ch = sup_shape[s % spb]
            slabs.append(np.ascontiguousarray(
                xflat[off:off + ch * P].reshape(ch, P, D).transpose(1, 0, 2)
            ).reshape(-1))
            off += ch * P
        xp = np.concatenate(slabs)
        bl = np.ascontiguousarray(blflat.reshape(nch, P).T)
        pad_counts.append(L - int(blk_cnt[core * blocks:(core + 1) * blocks].sum()))
        in_maps.append({"xp": xp, "bl": bl, "bln": -bl, "wrep": wrep,
                        "iota": iota})

    key = (blocks, blk_ch, kd, ka)
    if key not in _prog_cache:
        _prog_cache[key] = _build(*key)
    nc = _prog_cache[key]

    res = run_bass_kernel_spmd(nc, in_maps, list(range(n_cores))).results

    z_total = 0.0
    parts = []
    for core in range(n_cores):
        z_total += float(res[core]["zout"].astype(np.float64).sum()) - pad_counts[core]
        o = res[core]["outp"].astype(np.float32)
        parts.append(o.reshape(P, blocks, D).transpose(1, 0, 2)
                     .reshape(segs_per_core, D))
    out = np.concatenate(parts, axis=0)
    return (out / np.float32(z_total)).astype(np.float32)


def kernel(x, batch, W, b):
    x = np.asarray(x, np.float32)
    batch = np.asarray(batch)
    W = np.asarray(W, np.float32)
    return _pool(x, batch, W, num_graphs=16384)


if __name__ == "__main__":
    rng = np.random.default_rng(0)
    G = 1024
    n = 16000
    x = rng.standard_normal((n, D), dtype=np.float32)
    batch = np.sort(rng.integers(0, G, n)).astype(np.int64)
    W = (rng.standard_normal((D, 1), dtype=np.float32) / np.sqrt(D)).astype(np.float32)
    b = np.zeros((1,), np.float32)

    got = _pool(x, batch, W, num_graphs=G)

    s = (x @ W).ravel()
    a = np.exp(s - s.max()); a /= a.sum()
    want = np.zeros((G, D), np.float64)
    np.add.at(want, batch, x * a[:, None])
    want = want.astype(np.float32)
    num = np.abs(got - want).max()
    print("abs err:", num, "rel err:", num / np.abs(want).max())

